# revision 1
# baseline (speedup 1.0000x reference)
"""Bass kernel builder for ClassSeparationLossMargin.

Math: loss = mean_ij [ t*(1-cos) + (1-t)*relu(margin - (1-cos)) ]
  with cos = xn @ xn.T (row-normalized), t = same-class mask.

Device computes, per core (on a row-rolled copy of the full input so the
same SPMD program processes "rows 0:N/8" everywhere):
  G = H @ H.T with H = [xn | sqk*O]  (O = one-hot classes)  => G = cos + K*t
  relu_sum = weighted sum over scheduled tiles of relu(G + (margin-1))
  corr     = (0.9-K)*A - 2*B    A = sum_c n_c^2,  B = sum S^2, S = O.T @ xn
  out      = (relu_sum + corr/8 + dve_off) / N^2
Host sums the 8 outputs.

For same-class pairs relu(0.1+cos+K) = 0.1+cos+K exactly (K>0.9), so
sum relu-pass = C_diff + 0.1*A + B + K*A; want A - B + C_diff
=> corr = (0.9-K)*A - 2*B.  K = sqk^2 = 1.265625 (sqk=1.125 exact in bf16).

Triangle mode exploits G's symmetry: local row chunk r only processes col
chunks c with (c - r) mod T in {0..T/2}, weight 1 at the two ends and 2 in
the middle. Across the 8 rolled copies every unordered pair is counted
exactly twice (d = (C-R) mod T is roll-invariant), so w1 + 2*w2 = full sum.
"""

from contextlib import ExitStack

import numpy as np

import concourse.bacc as bacc
import concourse.mybir as mybir
import concourse.tile as tile
from concourse.masks import make_identity

F32 = mybir.dt.float32
BF16 = mybir.dt.bfloat16
I32 = mybir.dt.int32
OP = mybir.AluOpType
AF = mybir.ActivationFunctionType


def _consumer_schedule(N, P, RC, T, cw, triangle):
    """[(row_chunk, [(col_off, width)...], weight, engine)] with absolute
    hT columns; engine 'A' = scalar/ACT, 'D' = vector/DVE."""
    ops = []
    for r in range(RC):
        if triangle:
            half = T // 2
            ops.append((r, [(r * P, P), ((r + half) * P, P)], 1, None))
            span = (half - 1) * P
            off = (r + 1) * P
            while span > 0:
                w = min(cw, span)
                ops.append((r, [(off, w)], 2, None))
                off += w
                span -= w
        else:
            for c in range(0, N, cw):
                ops.append((r, [(c, cw)], 1, None))
    ta = td = 0.0
    out = []
    for (r, segs, w, _) in ops:
        fd = sum(s[1] for s in segs)
        ca = (172 + fd) / 1.2
        cd = (120 + fd) / 0.96
        if ta + ca <= td + cd:
            ta += ca
            out.append((r, segs, w, "A"))
        else:
            td += cd
            out.append((r, segs, w, "D"))
    return out


def build_nc(N=8192, D=64, C=17, margin=1.1, n_cores=8, cw=1024,
             triangle=True):
    """Inputs: b_t [128, T, D] f32 row-tiled, cm_t [128, T] i32.
    Output: out [1, 1] f32 partial loss."""
    sqk = 1.125
    K = sqk * sqk
    m1 = margin - 1.0            # 0.1
    P = 128
    T = N // P                   # row tiles of the full matrix
    E = D + 1                    # feature cols + norm col
    HD = D + C                   # Gram feature dim (81)
    RC = (N // n_cores) // P     # row chunks this core owns

    nc = bacc.Bacc("TRN2", target_bir_lowering=False, num_devices=n_cores)
    b_dram = nc.dram_tensor("b_t", [P, T, D], F32, kind="ExternalInput")
    cm_dram = nc.dram_tensor("cm_t", [P, T], I32, kind="ExternalInput")
    out_dram = nc.dram_tensor("out", [1, 1], F32, kind="ExternalOutput")

    sched = _consumer_schedule(N, P, RC, T, cw, triangle)
    n_a = {1: 0, 2: 0}
    n_d = {1: 0, 2: 0}
    for (_, segs, w, e) in sched:
        (n_a if e == "A" else n_d)[w] += 1
    # DVE accum quirk: accum = sum_f max(x, -m1) + s2(=0) -> under-counts m1
    # per element (doubled where the column weight is 2).
    dve_off = float(m1 * P * sum(
        sum(s[1] for s in segs) * w
        for (_, segs, w, e) in sched if e == "D"))

    with tile.TileContext(nc) as tc, ExitStack() as top:
        persist = top.enter_context(tc.tile_pool(name="persist", bufs=1))

        # ---- constants ----
        ident = persist.tile([P, P], BF16)
        make_identity(nc, ident[:])
        bias_m1 = persist.tile([P, 1], F32)
        nc.gpsimd.memset(bias_m1[:], m1)
        ones128 = persist.tile([P, 1], F32)
        nc.gpsimd.memset(ones128[:], 1.0)
        iota_i = persist.tile([P, T, C], I32)
        nc.gpsimd.iota(iota_i[:], pattern=[[0, T], [1, C]], base=0,
                       channel_multiplier=0)
        iotaf = persist.tile([P, T, C], F32)
        nc.vector.tensor_copy(iotaf[:], iota_i[:])

        # ---- inputs ----
        b_ext = persist.tile([P, T, E], F32)
        nc.sync.dma_start(b_ext[:, :, 0:D], b_dram[:])
        cm_i = persist.tile([P, T], I32)
        nc.sync.dma_start(cm_i[:], cm_dram[:])
        cm_f = persist.tile([P, T], F32)
        nc.vector.tensor_copy(cm_f[:], cm_i[:])

        # ---- normalization scalars ----
        sq_all = persist.tile([P, T, D], F32)
        nc.scalar.activation(sq_all[:], b_ext[:, :, 0:D], AF.Square)
        ns = persist.tile([P, T], F32)
        nc.vector.tensor_reduce(ns[:], sq_all[:], axis=mybir.AxisListType.X,
                                op=OP.add)
        # clamped norm lives in b_ext col D so (O_s.T @ b_ext)[c, D] = n_c
        norm = persist.tile([P, T], F32)
        nc.scalar.activation(norm[:], ns[:], AF.Sqrt)
        nc.vector.tensor_scalar(b_ext[:, :, D:E].squeeze(-1), norm[:],
                                1e-8, None, OP.max)
        s_til = persist.tile([P, T], F32)
        nc.vector.reciprocal(s_til[:], b_ext[:, :, D:E].squeeze(-1))

        # ---- bulk prep: onehot, O_s, xno = [xn | sqk*O] (bf16) ----
        onehot = persist.tile([P, T, C], F32)
        cm_b = cm_f[:, :, None].to_broadcast([P, T, C])
        nc.vector.tensor_tensor(onehot[:], iotaf[:], cm_b, OP.is_equal)
        o_s = persist.tile([P, T, C], F32)
        s_bc = s_til[:, :, None].to_broadcast([P, T, C])
        nc.gpsimd.tensor_tensor(o_s[:], onehot[:], s_bc, OP.mult)
        xno = persist.tile([P, T, HD], BF16)
        s_bd = s_til[:, :, None].to_broadcast([P, T, D])
        nc.gpsimd.tensor_tensor(xno[:, :, 0:D], b_ext[:, :, 0:D], s_bd,
                                OP.mult)
        nc.vector.tensor_scalar(xno[:, :, D:HD], onehot[:], sqk, None,
                                OP.mult)

        # ---- hT = xno^T via regular matmul against identity ----
        hT = persist.tile([HD, N], BF16)
        with tc.tile_pool(name="ps_a", bufs=3, space="PSUM") as ps_a:
            for g in range(T // 4):
                hps = ps_a.tile([HD, 4 * P], F32, tag="hps")
                for q in range(4):
                    t = 4 * g + q
                    nc.tensor.matmul(hps[:, q * P:(q + 1) * P], xno[:, t, :],
                                     ident[:], start=True, stop=True)
                cp = nc.scalar.copy if g % 2 == 0 else nc.vector.tensor_copy
                cp(hT[:, g * 4 * P:(g + 1) * 4 * P], hps[:])

        # ---- main loop ----
        nacc_a = max(n_a[1] + n_a[2], 1)
        nacc_d = max(n_d[1] + n_d[2], 1)
        acc_a = persist.tile([P, nacc_a], F32)
        acc_d = persist.tile([P, nacc_d], F32)
        wt_a = [0] * nacc_a
        wt_d = [0] * nacc_d
        with tc.tile_pool(name="ps_s", bufs=1, space="PSUM") as ps_s, \
             tc.tile_pool(name="ps_g", bufs=3, space="PSUM") as ps_g:
            ia = idv = 0
            for (r, segs, w, e) in sched:
                fd = sum(s[1] for s in segs)
                g = ps_g.tile([P, fd], F32,
                              tag="g" if fd > 2 * P else "g1")
                lhsT = hT[:, r * P:(r + 1) * P]
                x = 0
                for (off, width) in segs:
                    while width > 0:
                        mw = min(512, width)
                        nc.tensor.matmul(g[:, x:x + mw],
                                         lhsT, hT[:, off:off + mw],
                                         start=True, stop=True)
                        x += mw
                        off += mw
                        width -= mw
                if e == "A":
                    nc.scalar.activation(g[:], g[:], AF.Relu,
                                         bias=bias_m1[:, 0:1], scale=1.0,
                                         accum_out=acc_a[:, ia:ia + 1])
                    wt_a[ia] = w
                    ia += 1
                else:
                    nc.vector.tensor_scalar(g[:], g[:], -m1, 0.0,
                                            OP.max, OP.add,
                                            accum_out=acc_d[:, idv:idv + 1])
                    wt_d[idv] = w
                    idv += 1

            # ---- S_ext accumulation (PE tail; only corr depends on it) ----
            s_ps = ps_s.tile([C, E], F32)
            for t in range(T):
                nc.tensor.matmul(s_ps[:], o_s[:, t, :], b_ext[:, t, :],
                                 start=(t == 0), stop=(t == T - 1))

            # ---- corr from S_ext ----
            sqs = persist.tile([C, D], F32)
            b_vec = persist.tile([C, 1], F32)
            nc.scalar.activation(sqs[:], s_ps[:, 0:D], AF.Square,
                                 accum_out=b_vec[:])
            n2 = persist.tile([C, 1], F32)
            nc.scalar.activation(n2[:], s_ps[:, D:E], AF.Square)
            t1 = persist.tile([C, 1], F32)
            nc.vector.tensor_scalar(t1[:], b_vec[:], -2.0, None, OP.mult)
            corr_v = persist.tile([C, 1], F32)
            nc.vector.scalar_tensor_tensor(corr_v[:], n2[:], (1.0 - m1) - K,
                                           t1[:], OP.mult, OP.add)
            ones_c = persist.tile([C, 1], F32)
            nc.gpsimd.memset(ones_c[:], 1.0)

            # ---- final reduction ----
            # weighted column sums: since weights are only 1 or 2, reduce
            # per-weight column groups... columns are interleaved, so use a
            # weight vector via iota trick instead: build w vecs in python
            # as constant tiles is awkward; reduce in two passes with
            # strided APs is messy. Simplest: reduce all, plus reduce the
            # w==2 columns again (they are a prefix/suffix mix), via a
            # second accumulator approach below.
            red = persist.tile([P, 1], F32)
            red_a = persist.tile([P, 1], F32)
            red_d = persist.tile([P, 1], F32)
            nc.vector.tensor_reduce(red_a[:], acc_a[:],
                                    axis=mybir.AxisListType.X, op=OP.add)
            nc.vector.tensor_reduce(red_d[:], acc_d[:],
                                    axis=mybir.AxisListType.X, op=OP.add)
            nc.vector.tensor_add(red[:], red_a[:], red_d[:])
            with tc.tile_pool(name="ps_f", bufs=2, space="PSUM") as ps_f:
                corr_ps = ps_f.tile([1, 1], F32, tag="corr")
                nc.tensor.matmul(corr_ps[:], corr_v[:], ones_c[:],
                                 start=True, stop=True)
                tot_ps = ps_f.tile([1, 1], F32, tag="tot")
                nc.tensor.matmul(tot_ps[:], red[:], ones128[:],
                                 start=True, stop=True)
                f1 = persist.tile([1, 1], F32)
                nc.vector.tensor_scalar(f1[:], corr_ps[:], 1.0 / n_cores,
                                        None, OP.mult)
                f2 = persist.tile([1, 1], F32)
                nc.vector.scalar_tensor_tensor(f2[:], tot_ps[:], dve_off,
                                               f1[:], OP.add, OP.add)
                fin = persist.tile([1, 1], F32)
                nc.vector.tensor_scalar(fin[:], f2[:],
                                        1.0 / (float(N) * N), None, OP.mult)
                nc.sync.dma_start(out_dram[:], fin[:])

    nc.compile()
    return nc, dict(N=N, T=T, n_cores=n_cores, sched=sched)


def host_inputs(bottleneck, class_map, n_cores=8):
    """Full inputs -> per-core in_maps (rolled + tiled layouts)."""
    N, D = bottleneck.shape
    P = 128
    T = N // P
    roll = N // n_cores
    maps = []
    for c in range(n_cores):
        b = np.roll(bottleneck, -roll * c, axis=0)
        cm = np.roll(class_map, -roll * c, axis=0)
        b_t = np.ascontiguousarray(
            b.reshape(T, P, D).transpose(1, 0, 2))          # [128, T, D]
        cm_t = np.ascontiguousarray(cm.reshape(T, P).T)     # [128, T]
        maps.append({"b_t": b_t.astype(np.float32),
                     "cm_t": cm_t.astype(np.int32)})
    return maps


# ---------------------------------------------------------------------------
# Harness entry point: kernel(**inputs) takes the FULL unsharded inputs and
# returns the full (scalar) output. Shards by row-rolling across 8 cores,
# runs the SPMD Bass kernel, and sums the per-core partials on the host.
# ---------------------------------------------------------------------------
from concourse.bass_utils import run_bass_kernel_spmd

_CACHED = {}


def _get_nc():
    if "nc" not in _CACHED:
        _CACHED["nc"] = build_nc(N=8192, D=64, C=17, margin=1.1, n_cores=8,
                                 cw=1024, triangle=True)[0]
    return _CACHED["nc"]


def kernel(bottleneck, class_map):
    bottleneck = np.asarray(bottleneck, dtype=np.float32)
    class_map = np.asarray(class_map, dtype=np.int32)
    nc = _get_nc()
    maps = host_inputs(bottleneck, class_map, n_cores=8)
    res = run_bass_kernel_spmd(nc, maps, core_ids=list(range(8)))
    total = sum(float(r["out"][0, 0]) for r in res.results)
    return np.float32(total)
